# revision 9
# baseline (speedup 1.0000x reference)
"""FP4Net (bnb-FP4 quantize-dequantize 4-layer MLP) Trainium2 kernel.

Strategy (8 NeuronCores):
  - Data-parallel over batch for the matmuls: each core handles 1024 of 8192 rows.
  - FP4 quant-dequant of the weights is sharded 8x across cores (by output-row
    blocks, keeping the 64-elem FP4 blocks intact), computed exactly with fp32
    bit tricks on the vector engine, stored transposed (W.T layout) in fp16,
    then AllGathered so every core has all dequantized weights.
  - Each weight's AllGather is split into chunks of 128-row blocks; chunk ci
    gathers j-tiles {c*nrt+r for cores c, r in chunk}, and each layer's j-loop
    walks chunks in order, so matmuls start as soon as the first chunk lands.
    (w1: 2 chunks for an early start, w2: 4 to make the l2 deadline, w3: 2,
    w4: 1.)
  - The dequant output accumulates per 128-row block in SBUF and is transposed
    by ONE coarse DMA-transpose per block (13 total): the tile scheduler
    mutually excludes XBAR DMA-transposes and collectives, so many fine
    transposes interleaved with AllGathers would interlock the pipeline.
  - Queue discipline: Vector = dequant math only; Scalar(ACT) = dequant input
    loads + block transposes + x/bias staging + epilogues; GpSimd = stores +
    AllGathers; SP(sync) = weight-strip loads + output stores (pure PE feed).
  - Dequant-side work is emitted into the layer loops by estimated ready time,
    so no queue head-of-line blocks on a dependency that isn't about to be
    satisfied.

Rounding trick: with g = 3*w/scale, the bnb FP4 codebook {0, 1/192, 1/6, 1/4,
1/3, 1/2, 2/3, 1} maps to {0, 1/64, 1/2, 3/4, 1, 3/2, 2, 3}: round-to-nearest
over that set == round g to 1 stored mantissa bit (round-half-up via exact
small-significand integer adds), clamped below at 1/2, plus a two-threshold
step for the {0, 1/64} region. Verified bit-exact vs the jax reference modulo
~1-ulp boundary fuzz (~1 flipped element per 16M weights on the actual data).
"""
import sys
import numpy as np

for _p in ("/opt/trn_rl_repo", "/root/.axon_site/_ro/trn_rl_repo"):
    if _p not in sys.path:
        sys.path.append(_p)

N_CORES = 8
B, IN, H, OUT = 8192, 1024, 4096, 1024
BS = B // N_CORES          # batch shard per core
HS = H // N_CORES          # hidden-row shard per core (w1/w2/w3)
OS = OUT // N_CORES        # out-row shard per core (w4)

# FP4 codebook-derived threshold constants (g-space = 3*norm), f64 precision
_FP4_POS = np.array([0.0, 0.0052083333, 0.6666667, 1.0, 0.3333333, 0.5,
                     0.1666667, 0.25], dtype=np.float32)
_CS = np.sort(_FP4_POS).astype(np.float64)
_TL = float(np.float32(3.0 * (_CS[0] + _CS[1]) / 2.0))
_TH = float(np.float32(3.0 * (_CS[1] + _CS[2]) / 2.0))
LO_BITS = int(np.float32(1.0 / 64).view(np.uint32))   # 0x3C800000
BIG_BITS = 0x40400000                                  # bits of 3.0


def _i32(x):
    return int(np.uint32(x).view(np.int32))


_CACHED = {}

# weight dims per layer: (rows of W == dout, k == contraction)
WDIMS = {1: (H, IN), 2: (H, H), 3: (H, H), 4: (OUT, H)}
NRT = {l: (d // N_CORES) // 128 for l, (d, _k) in WDIMS.items()}  # r-blocks
CHUNKS = {1: [[0, 1], [2, 3]], 2: [[0], [1], [2], [3]],
          3: [[0, 1], [2, 3]], 4: [[0]]}      # r-blocks per AllGather chunk
FDQ = 512          # dequant tile free-size (fp32 elems per partition)
NBQ = FDQ // 64    # fp4 blocks per tile

# --- static timing model (us) used only to order emission ---
DVE_TILE_US = 6.9          # dequant DVE time per [128, FDQ] tile
DVE_T0_US = 7.0            # engine init before first dequant op
MM_US = 0.263              # per N=512 matmul at 13/16 clock
L1_START_US = 85.0         # estimated first-epilogue time of layer 1
EV_MARGIN_US = 3.0


def _build_nc(taps=False):
    import concourse.bass as bass
    import concourse.mybir as mybir
    import concourse.tile as tile
    from concourse import bacc

    dt = mybir.dt
    Alu = mybir.AluOpType
    Act = mybir.ActivationFunctionType

    nc = bacc.Bacc("TRN2", target_bir_lowering=False, debug=False,
                   num_devices=N_CORES)

    # ---- I/O ----
    xs = nc.dram_tensor("xst", [IN, BS], dt.float16, kind="ExternalInput")
    w_in = {
        1: nc.dram_tensor("w1s", [HS, IN], dt.float32, kind="ExternalInput"),
        2: nc.dram_tensor("w2s", [HS, H], dt.float32, kind="ExternalInput"),
        3: nc.dram_tensor("w3s", [HS, H], dt.float32, kind="ExternalInput"),
        4: nc.dram_tensor("w4s", [OS, H], dt.float32, kind="ExternalInput"),
    }
    b_in = {
        1: nc.dram_tensor("b1", [128, H // 128], dt.float32, kind="ExternalInput"),
        2: nc.dram_tensor("b2", [128, H // 128], dt.float32, kind="ExternalInput"),
        3: nc.dram_tensor("b3", [128, H // 128], dt.float32, kind="ExternalInput"),
        4: nc.dram_tensor("b4", [128, OUT // 128], dt.float32, kind="ExternalInput"),
    }
    y_out = nc.dram_tensor("y", [OUT, BS], dt.float32, kind="ExternalOutput")

    # ---- internal DRAM: per-chunk shard + AllGather outputs ----
    dq_shard = {}   # (l, ci) -> local chunk tensor [len(chunk), K, 128]
    dq_chunk = {}   # (l, ci) -> gathered [N_CORES*len(chunk), K, 128]
    for l, (dout, k) in WDIMS.items():
        for ci, chunk in enumerate(CHUNKS[l]):
            dq_shard[(l, ci)] = nc.dram_tensor(
                f"dqs{l}{ci}", [len(chunk), k, 128], dt.float16)
            dq_chunk[(l, ci)] = nc.dram_tensor(
                f"dqc{l}{ci}", [N_CORES * len(chunk), k, 128], dt.float16,
                addr_space="Shared")

    with tile.TileContext(nc) as tc:
        with (
            tc.tile_pool(name="const", bufs=1) as cpool,
            tc.tile_pool(name="bias", bufs=1) as bpool,
            tc.tile_pool(name="a0", bufs=1) as a0pool,
            tc.tile_pool(name="acts", bufs=2) as apool,
            tc.tile_pool(name="dqin", bufs=3) as dqin_pool,
            tc.tile_pool(name="dqtmp", bufs=1) as dqtmp_pool,
            tc.tile_pool(name="dqblk", bufs=2) as dqblk_pool,
            tc.tile_pool(name="dqtp", bufs=4) as dqtp_pool,
            tc.tile_pool(name="wt", bufs=3) as wpool,
            tc.tile_pool(name="psum", bufs=8, space="PSUM") as pspool,
        ):
            # int32 constants for scalar_tensor_tensor scalars
            c_half = cpool.tile([128, 1], dt.int32)
            nc.vector.memset(c_half[:], _i32(0x3F000000))
            c_sign = cpool.tile([128, 1], dt.int32)
            nc.vector.memset(c_sign[:], _i32(0x80000000))

            # ---- biases -> SBUF [128, ntiles]; x -> SBUF a0 (Scalar queue) ----
            b_sb = {}
            for l, (dout, _k) in WDIMS.items():
                nj = dout // 128
                bt = bpool.tile([128, nj], dt.float32, tag=f"bias{l}")
                nc.scalar.dma_start(bt[:], b_in[l][:])
                b_sb[l] = bt
            a_cur = a0pool.tile([128, IN // 128, BS], dt.float16)
            nc.scalar.dma_start(
                a_cur[:], xs[:].rearrange("(i p) b -> p i b", p=128))

            # ---- dequant pipeline pieces ----
            # tile order: (l, r, cix) over all weights, r-block-major
            all_tiles = []
            for l in WDIMS:
                for r in range(NRT[l]):
                    for cix in range(WDIMS[l][1] // FDQ):
                        all_tiles.append((l, r, cix))
            t_index = {t: i for i, t in enumerate(all_tiles)}

            def dve_done(i):  # estimated DVE completion time of tile i (us)
                return DVE_T0_US + (i + 1) * DVE_TILE_US

            def emit_dq_load(l, r, cix):
                """Scalar queue: stage [128 rows, FDQ] fp32 of weight l."""
                w = dqin_pool.tile([128, NBQ, 64], dt.float32, tag="dqw")
                nc.scalar.dma_start(
                    w[:],
                    w_in[l][r * 128:(r + 1) * 128, cix * FDQ:(cix + 1) * FDQ]
                    .rearrange("p (b i) -> p b i", i=64))
                return w

            blk_sb = {}   # (l, r) -> accumulating [128, K/64, 64] fp16 buffer

            def emit_dq_vec(l, r, cix, w):
                """Vector queue: dequant math -> slice of the r-block buffer."""
                if cix == 0:
                    blk = dqblk_pool.tile([128, WDIMS[l][1] // 64, 64],
                                          dt.float16, tag="dqblk")
                    blk_sb[(l, r)] = blk
                scale = dqtmp_pool.tile([128, NBQ, 1], dt.float32, tag="scale")
                nc.vector.tensor_reduce(scale[:], w[:], axis=mybir.AxisListType.X,
                                        op=Alu.max, apply_absolute_value=True)
                recip = dqtmp_pool.tile([128, NBQ, 1], dt.float32, tag="recip")
                nc.vector.reciprocal(recip[:], scale[:])
                s3 = dqtmp_pool.tile([128, NBQ, 1], dt.float32, tag="s3")
                nc.vector.tensor_scalar_mul(s3[:], scale[:], 1.0 / 3.0)
                g = dqtmp_pool.tile([128, NBQ, 64], dt.float32, tag="g")
                nc.vector.scalar_tensor_tensor(
                    g[:], w[:], 3.0, recip[:].broadcast_to((128, NBQ, 64)),
                    op0=Alu.mult, op1=Alu.mult)
                gi = g[:].bitcast(dt.int32)
                # NOTE: DVE ops must never write in-place onto their own input
                # (dual-port perf modes race), and int adds must keep few
                # significant bits (the int ALU path is fp32-internal).
                ta = dqtmp_pool.tile([128, NBQ, 64], dt.int32, tag="ta")
                nc.vector.tensor_scalar(ta[:], gi, _i32(0x7FFFFFFF), None,
                                        op0=Alu.bitwise_and)  # m0 = |g| bits
                tb = dqtmp_pool.tile([128, NBQ, 64], dt.int32, tag="tb")
                nc.vector.tensor_scalar(tb[:], ta[:], _i32(0xFFC00000), None,
                                        op0=Alu.bitwise_and)  # trunc
                tc_ = dqtmp_pool.tile([128, NBQ, 64], dt.int32, tag="tc")
                nc.vector.tensor_scalar(tc_[:], ta[:], _i32(0x00200000), _i32(1),
                                        op0=Alu.bitwise_and,
                                        op1=Alu.logical_shift_left)  # half-bit<<1
                te = dqtmp_pool.tile([128, NBQ, 64], dt.int32, tag="te")
                nc.vector.tensor_tensor(te[:], tb[:], tc_[:],
                                        op=Alu.add)  # r2a (exact: 10+1 sig bits)
                af = ta[:].bitcast(dt.float32)  # |g| as float
                # M1L = (|g|>TL)*LO_BITS, M2B = (|g|>TH)*BIG_BITS -- the float
                # products are exact (consts have <=5 significant bits)
                nc.vector.tensor_scalar(tb[:], af, _TL, float(LO_BITS),
                                        op0=Alu.is_gt, op1=Alu.mult)
                td = dqtmp_pool.tile([128, NBQ, 64], dt.int32, tag="td")
                nc.vector.tensor_scalar(td[:], af, _TH, float(BIG_BITS),
                                        op0=Alu.is_gt, op1=Alu.mult)
                nc.vector.tensor_tensor(tc_[:], tb[:], td[:],
                                        op=Alu.add)  # sel (disjoint bits)
                nc.vector.scalar_tensor_tensor(
                    tb[:], te[:], c_half[:], tc_[:],
                    op0=Alu.max, op1=Alu.min)  # mag
                nc.vector.scalar_tensor_tensor(
                    ta[:], gi, c_sign[:], tb[:],
                    op0=Alu.bitwise_and, op1=Alu.bitwise_or)  # signed
                blk = blk_sb[(l, r)]
                nc.vector.tensor_tensor(
                    blk[:, cix * NBQ:(cix + 1) * NBQ, :],
                    ta[:].bitcast(dt.float32),
                    s3[:].broadcast_to((128, NBQ, 64)), op=Alu.mult)

            def emit_dq_tp(l, ci, r):
                """Scalar: per-tile transposes for the whole 128-row block,
                batched; GpSimd: stores to the chunk shard."""
                K = WDIMS[l][1]
                blk = blk_sb.pop((l, r))
                ri = r - CHUNKS[l][ci][0]
                for cix in range(K // FDQ):
                    dqt = dqtp_pool.tile([128, FDQ // 128, 128], dt.float16,
                                         tag="dqt")
                    nc.scalar.dma_start_transpose(
                        dqt[:],
                        blk[:, cix * NBQ:(cix + 1) * NBQ, :]
                        .rearrange("p b i -> p (b i)"))
                    nc.gpsimd.dma_start(
                        dq_shard[(l, ci)][ri, cix * FDQ:(cix + 1) * FDQ, :]
                        .rearrange("(c p) h -> p c h", p=128),
                        dqt[:])

            def emit_ag(l, ci):
                nc.gpsimd.collective_compute(
                    "AllGather", Alu.bypass,
                    replica_groups=[list(range(N_CORES))],
                    ins=[dq_shard[(l, ci)][:]],
                    outs=[dq_chunk[(l, ci)][:]],
                )

            # ---- event list: (ready_us, seq, kind, payload), ready-ordered ----
            events = []
            seq = 0
            ntk = {l: WDIMS[l][1] // FDQ for l in WDIMS}
            for t in all_tiles:
                i = t_index[t]
                rdy_load = 0.0 if i < 3 else dve_done(i - 3)
                events.append((rdy_load, seq, "load", t)); seq += 1
                events.append((rdy_load, seq, "vec", t)); seq += 1
                l, r, cix = t
                if cix == ntk[l] - 1:
                    ci = next(c for c, ch in enumerate(CHUNKS[l]) if r in ch)
                    events.append((dve_done(i) + EV_MARGIN_US, seq, "tp",
                                   (l, ci, r))); seq += 1
                    if r == CHUNKS[l][ci][-1]:
                        events.append((dve_done(i) + EV_MARGIN_US, seq, "ag",
                                       (l, ci)))
                        seq += 1
            events.sort(key=lambda e: (e[0], e[1]))
            events = events[::-1]  # pop from end

            staged = {}   # tile -> dqin handle

            def flush_events(now_us):
                while events and events[-1][0] <= now_us:
                    _, _, kind, payload = events.pop()
                    if kind == "load":
                        staged[payload] = emit_dq_load(*payload)
                    elif kind == "vec":
                        emit_dq_vec(*payload, staged.pop(payload))
                    elif kind == "tp":
                        emit_dq_tp(*payload)
                    else:
                        emit_ag(*payload)

            # everything ready before the first epilogue goes out up front
            flush_events(L1_START_US)

            # ---- matmul layers, chunk-ordered j loops ----
            t_us = L1_START_US
            for l, (dout, K) in WDIMS.items():
                nj = dout // 128
                nk = K // 128
                nrt = NRT[l]
                half = nk // 2
                out_dt = dt.float32 if l == 4 else dt.float16
                a_next = apool.tile([128, nj, BS], out_dt, tag="acts")
                j_period = nk * (BS // 512) * MM_US
                for ci, chunk in enumerate(CHUNKS[l]):
                    for c in range(N_CORES):
                        for ri, r in enumerate(chunk):
                            j = c * nrt + r
                            src = dq_chunk[(l, ci)][c * len(chunk) + ri]
                            wts = []
                            for i0 in (0, half):
                                wt_h = wpool.tile([128, half, 128], dt.float16,
                                                  tag="wt")
                                nc.sync.dma_start(
                                    wt_h[:],
                                    src[i0 * 128:(i0 + half) * 128, :]
                                    .rearrange("(i p) h -> p i h", p=128))
                                wts.append(wt_h)
                            ps = []
                            for _n in range(BS // 512):
                                ps_t = pspool.tile([128, 512], dt.float32,
                                                   tag="ps")
                                ps.append(ps_t)
                            for i in range(nk):
                                for n in range(BS // 512):
                                    nc.tensor.matmul(
                                        ps[n][:], wts[i // half][:, i % half, :],
                                        a_cur[:, i, n * 512:(n + 1) * 512],
                                        start=(i == 0), stop=(i == nk - 1))
                            t_us += j_period
                            flush_events(t_us)
                            act_fn = Act.Sigmoid if l == 4 else Act.Relu
                            for n in range(BS // 512):
                                nc.scalar.activation(
                                    a_next[:, j, n * 512:(n + 1) * 512],
                                    ps[n][:], act_fn,
                                    bias=b_sb[l][:, j:j + 1], scale=1.0)
                            if l == 4:
                                nc.sync.dma_start(
                                    y_out[j * 128:(j + 1) * 128, :],
                                    a_next[:, j, :])
                a_cur = a_next
            flush_events(1e9)

    nc.compile()
    return nc


def _get_nc():
    if "nc" not in _CACHED:
        _CACHED["nc"] = _build_nc()
    return _CACHED["nc"]


def build_in_maps(inputs):
    x = np.asarray(inputs["x"], dtype=np.float32)
    ws = {l: np.ascontiguousarray(np.asarray(inputs[f"w{l}"], dtype=np.float32))
          for l in (1, 2, 3, 4)}
    bs = {l: np.ascontiguousarray(
        np.asarray(inputs[f"b{l}"], dtype=np.float32).reshape(-1, 128).T)
        for l in (1, 2, 3, 4)}
    in_maps = []
    for c in range(N_CORES):
        m = {
            "xst": np.ascontiguousarray(
                x[c * BS:(c + 1) * BS].T.astype(np.float16)),
            "w1s": ws[1][c * HS:(c + 1) * HS],
            "w2s": ws[2][c * HS:(c + 1) * HS],
            "w3s": ws[3][c * HS:(c + 1) * HS],
            "w4s": ws[4][c * OS:(c + 1) * OS],
            "b1": bs[1], "b2": bs[2], "b3": bs[3], "b4": bs[4],
        }
        in_maps.append(m)
    return in_maps


def kernel(**inputs):
    from concourse.bass_utils import run_bass_kernel_spmd

    nc = _get_nc()
    in_maps = build_in_maps(inputs)
    res = run_bass_kernel_spmd(nc, in_maps, list(range(N_CORES)))
    out = np.empty((B, OUT), dtype=np.float32)
    for c in range(N_CORES):
        out[c * BS:(c + 1) * BS] = res.results[c]["y"].T
    return out


if __name__ == "__main__":
    rng = np.random.default_rng(0)
    ins = {
        "x": rng.standard_normal((B, IN)).astype(np.float32),
        "w1": (rng.standard_normal((H, IN)) * 0.1).astype(np.float32),
        "b1": np.zeros(H, np.float32),
        "w2": (rng.standard_normal((H, H)) * 0.1).astype(np.float32),
        "b2": np.zeros(H, np.float32),
        "w3": (rng.standard_normal((H, H)) * 0.1).astype(np.float32),
        "b3": np.zeros(H, np.float32),
        "w4": (rng.standard_normal((OUT, H)) * 0.1).astype(np.float32),
        "b4": np.zeros(OUT, np.float32),
    }
    y = kernel(**ins)
    print("kernel ran, output shape", y.shape, "mean", float(y.mean()))


# revision 14
# speedup vs baseline: 1.1902x; 1.1902x over previous
"""FP4Net (bnb-FP4 quantize-dequantize 4-layer MLP) Trainium2 kernel.

Strategy (8 NeuronCores):
  - Data-parallel over batch for the matmuls: each core handles 1024 of 8192 rows.
  - FP4 quant-dequant of the weights is sharded 8x across cores (by output-row
    blocks, keeping the 64-elem FP4 blocks intact), computed exactly with fp32
    bit tricks on the vector engine, stored transposed (W.T layout) in fp16,
    then AllGathered so every core has all dequantized weights.
  - Each weight's AllGather is split into chunks of 128-row blocks; chunk ci
    gathers j-tiles {c*nrt+r for cores c, r in chunk}, and each layer's j-loop
    walks chunks in order, so matmuls start as soon as the first chunk lands.
    (w1: 2 chunks for an early start, w2: 4 to make the l2 deadline, w3: 2,
    w4: 1.)
  - The dequant output accumulates per 128-row block in SBUF and is transposed
    by ONE coarse DMA-transpose per block (13 total): the tile scheduler
    mutually excludes XBAR DMA-transposes and collectives, so many fine
    transposes interleaved with AllGathers would interlock the pipeline.
  - Queue discipline: Vector = dequant math only; Scalar(ACT) = dequant input
    loads + block transposes + x/bias staging + epilogues; GpSimd = stores +
    AllGathers; SP(sync) = weight-strip loads + output stores (pure PE feed).
  - Dequant-side work is emitted into the layer loops by estimated ready time,
    so no queue head-of-line blocks on a dependency that isn't about to be
    satisfied.

Rounding trick: with g = 3*w/scale, the bnb FP4 codebook {0, 1/192, 1/6, 1/4,
1/3, 1/2, 2/3, 1} maps to {0, 1/64, 1/2, 3/4, 1, 3/2, 2, 3}: round-to-nearest
over that set == round g to 1 stored mantissa bit (round-half-up via exact
small-significand integer adds), clamped below at 1/2, plus a two-threshold
step for the {0, 1/64} region. Verified bit-exact vs the jax reference modulo
~1-ulp boundary fuzz (~1 flipped element per 16M weights on the actual data).
"""
import sys
import numpy as np

for _p in ("/opt/trn_rl_repo", "/root/.axon_site/_ro/trn_rl_repo"):
    if _p not in sys.path:
        sys.path.append(_p)

N_CORES = 8
B, IN, H, OUT = 8192, 1024, 4096, 1024
BS = B // N_CORES          # batch shard per core
HS = H // N_CORES          # hidden-row shard per core (w1/w2/w3)
OS = OUT // N_CORES        # out-row shard per core (w4)

# FP4 codebook-derived threshold constants (g-space = 3*norm), f64 precision
_FP4_POS = np.array([0.0, 0.0052083333, 0.6666667, 1.0, 0.3333333, 0.5,
                     0.1666667, 0.25], dtype=np.float32)
_CS = np.sort(_FP4_POS).astype(np.float64)
_TL = float(np.float32(3.0 * (_CS[0] + _CS[1]) / 2.0))
_TH = float(np.float32(3.0 * (_CS[1] + _CS[2]) / 2.0))
LO_BITS = int(np.float32(1.0 / 64).view(np.uint32))   # 0x3C800000
BIG_BITS = 0x40400000                                  # bits of 3.0


def _i32(x):
    return int(np.uint32(x).view(np.int32))


_CACHED = {}

# weight dims per layer: (rows of W == dout, k == contraction)
WDIMS = {1: (H, IN), 2: (H, H), 3: (H, H), 4: (OUT, H)}
NRT = {l: (d // N_CORES) // 128 for l, (d, _k) in WDIMS.items()}  # r-blocks
CHUNKS = {1: [[0], [1], [2], [3]], 2: [[0], [1], [2], [3]],
          3: [[0, 1], [2, 3]], 4: [[0]]}      # r-blocks per AllGather chunk
FDQ = 512          # dequant tile free-size (fp32 elems per partition)
NBQ = FDQ // 64    # fp4 blocks per tile

# --- static timing model (us) used only to order emission ---
DVE_TILE_US = 6.9          # dequant DVE time per [128, FDQ] tile
DVE_T0_US = 7.0            # engine init before first dequant op
MM_US = 0.263              # per N=512 matmul at 13/16 clock
L1_START_US = 85.0         # estimated first-epilogue time of layer 1
EV_MARGIN_US = 3.0


def _build_nc(taps=False):
    import concourse.bass as bass
    import concourse.mybir as mybir
    import concourse.tile as tile
    from concourse import bacc

    dt = mybir.dt
    Alu = mybir.AluOpType
    Act = mybir.ActivationFunctionType

    nc = bacc.Bacc("TRN2", target_bir_lowering=False, debug=False,
                   num_devices=N_CORES)

    # ---- I/O ----
    xs = nc.dram_tensor("xst", [IN, BS], dt.float16, kind="ExternalInput")
    w_in = {
        1: nc.dram_tensor("w1s", [HS, IN], dt.float32, kind="ExternalInput"),
        2: nc.dram_tensor("w2s", [HS, H], dt.float32, kind="ExternalInput"),
        3: nc.dram_tensor("w3s", [HS, H], dt.float32, kind="ExternalInput"),
        4: nc.dram_tensor("w4s", [OS, H], dt.float32, kind="ExternalInput"),
    }
    b_in = {
        1: nc.dram_tensor("b1", [128, H // 128], dt.float32, kind="ExternalInput"),
        2: nc.dram_tensor("b2", [128, H // 128], dt.float32, kind="ExternalInput"),
        3: nc.dram_tensor("b3", [128, H // 128], dt.float32, kind="ExternalInput"),
        4: nc.dram_tensor("b4", [128, OUT // 128], dt.float32, kind="ExternalInput"),
    }
    y_out = nc.dram_tensor("y", [OUT, BS], dt.float32, kind="ExternalOutput")

    # ---- internal DRAM: per-chunk shard + AllGather outputs ----
    dq_shard = {}   # (l, ci) -> local chunk tensor [len(chunk), K, 128]
    dq_chunk = {}   # (l, ci) -> gathered [N_CORES*len(chunk), K, 128]
    for l, (dout, k) in WDIMS.items():
        for ci, chunk in enumerate(CHUNKS[l]):
            dq_shard[(l, ci)] = nc.dram_tensor(
                f"dqs{l}{ci}", [len(chunk), k, 128], dt.float16)
            dq_chunk[(l, ci)] = nc.dram_tensor(
                f"dqc{l}{ci}", [N_CORES * len(chunk), k, 128], dt.float16,
                addr_space="Shared")

    with tile.TileContext(nc) as tc:
        with (
            tc.tile_pool(name="const", bufs=1) as cpool,
            tc.tile_pool(name="bias", bufs=1) as bpool,
            tc.tile_pool(name="a0", bufs=1) as a0pool,
            tc.tile_pool(name="acts", bufs=2) as apool,
            tc.tile_pool(name="dqin", bufs=3) as dqin_pool,
            tc.tile_pool(name="dqtmp", bufs=1) as dqtmp_pool,
            tc.tile_pool(name="dqout", bufs=4) as dqout_pool,
            tc.tile_pool(name="stp", bufs=4) as stp_pool,
            tc.tile_pool(name="wt", bufs=4) as wpool,
            tc.tile_pool(name="psum", bufs=8, space="PSUM") as pspool,
        ):
            # int32 constants for scalar_tensor_tensor scalars
            c_half = cpool.tile([128, 1], dt.int32)
            nc.vector.memset(c_half[:], _i32(0x3F000000))
            c_sign = cpool.tile([128, 1], dt.int32)
            nc.vector.memset(c_sign[:], _i32(0x80000000))

            # ---- biases -> SBUF [128, ntiles]; x -> SBUF a0 (Scalar queue) ----
            b_sb = {}
            for l, (dout, _k) in WDIMS.items():
                nj = dout // 128
                bt = bpool.tile([128, nj], dt.float32, tag=f"bias{l}")
                nc.scalar.dma_start(bt[:], b_in[l][:])
                b_sb[l] = bt
            a_cur = a0pool.tile([128, IN // 128, BS], dt.float16)
            nc.scalar.dma_start(
                a_cur[:], xs[:].rearrange("(i p) b -> p i b", p=128))

            # ---- dequant pipeline pieces ----
            # tile order: (l, r, cix) over all weights, r-block-major
            all_tiles = []
            for l in WDIMS:
                for r in range(NRT[l]):
                    for cix in range(WDIMS[l][1] // FDQ):
                        all_tiles.append((l, r, cix))
            t_index = {t: i for i, t in enumerate(all_tiles)}

            def dve_done(i):  # estimated DVE completion time of tile i (us)
                return DVE_T0_US + (i + 1) * DVE_TILE_US

            def emit_dq_load(l, r, cix):
                """Scalar queue: stage [128 rows, FDQ] fp32 of weight l."""
                w = dqin_pool.tile([128, NBQ, 64], dt.float32, tag="dqw")
                nc.scalar.dma_start(
                    w[:],
                    w_in[l][r * 128:(r + 1) * 128, cix * FDQ:(cix + 1) * FDQ]
                    .rearrange("p (b i) -> p b i", i=64))
                return w

            def emit_dq_vec(l, r, cix, w):
                """Vector queue: dequant math + 32x32 stream-transpose."""
                scale = dqtmp_pool.tile([128, NBQ, 1], dt.float32, tag="scale")
                nc.vector.tensor_reduce(scale[:], w[:], axis=mybir.AxisListType.X,
                                        op=Alu.max, apply_absolute_value=True)
                recip = dqtmp_pool.tile([128, NBQ, 1], dt.float32, tag="recip")
                nc.vector.reciprocal(recip[:], scale[:])
                s3 = dqtmp_pool.tile([128, NBQ, 1], dt.float32, tag="s3")
                nc.vector.tensor_scalar_mul(s3[:], scale[:], 1.0 / 3.0)
                g = dqtmp_pool.tile([128, NBQ, 64], dt.float32, tag="g")
                nc.vector.scalar_tensor_tensor(
                    g[:], w[:], 3.0, recip[:].broadcast_to((128, NBQ, 64)),
                    op0=Alu.mult, op1=Alu.mult)
                gi = g[:].bitcast(dt.int32)
                # NOTE: DVE ops must never write in-place onto their own input
                # (dual-port perf modes race), and int adds must keep few
                # significant bits (the int ALU path is fp32-internal).
                ta = dqtmp_pool.tile([128, NBQ, 64], dt.int32, tag="ta")
                nc.vector.tensor_scalar(ta[:], gi, _i32(0x7FFFFFFF), None,
                                        op0=Alu.bitwise_and)  # m0 = |g| bits
                tb = dqtmp_pool.tile([128, NBQ, 64], dt.int32, tag="tb")
                nc.vector.tensor_scalar(tb[:], ta[:], _i32(0xFFC00000), None,
                                        op0=Alu.bitwise_and)  # trunc
                tc_ = dqtmp_pool.tile([128, NBQ, 64], dt.int32, tag="tc")
                nc.vector.tensor_scalar(tc_[:], ta[:], _i32(0x00200000), _i32(1),
                                        op0=Alu.bitwise_and,
                                        op1=Alu.logical_shift_left)  # half-bit<<1
                te = dqtmp_pool.tile([128, NBQ, 64], dt.int32, tag="te")
                nc.vector.tensor_tensor(te[:], tb[:], tc_[:],
                                        op=Alu.add)  # r2a (exact: 10+1 sig bits)
                af = ta[:].bitcast(dt.float32)  # |g| as float
                # M1L = (|g|>TL)*LO_BITS, M2B = (|g|>TH)*BIG_BITS -- the float
                # products are exact (consts have <=5 significant bits)
                nc.vector.tensor_scalar(tb[:], af, _TL, float(LO_BITS),
                                        op0=Alu.is_gt, op1=Alu.mult)
                td = dqtmp_pool.tile([128, NBQ, 64], dt.int32, tag="td")
                nc.vector.tensor_scalar(td[:], af, _TH, float(BIG_BITS),
                                        op0=Alu.is_gt, op1=Alu.mult)
                nc.vector.tensor_tensor(tc_[:], tb[:], td[:],
                                        op=Alu.add)  # sel (disjoint bits)
                nc.vector.scalar_tensor_tensor(
                    tb[:], te[:], c_half[:], tc_[:],
                    op0=Alu.max, op1=Alu.min)  # mag
                nc.vector.scalar_tensor_tensor(
                    ta[:], gi, c_sign[:], tb[:],
                    op0=Alu.bitwise_and, op1=Alu.bitwise_or)  # signed
                dq = dqout_pool.tile([128, NBQ, 64], dt.float16, tag="dq")
                nc.vector.tensor_tensor(
                    dq[:], ta[:].bitcast(dt.float32),
                    s3[:].broadcast_to((128, NBQ, 64)), op=Alu.mult)
                # 32x32 block transpose on the DVE reshape front-end:
                # st[32a+u, 32b+v] = dq[32a+v, 32b+u]
                st = stp_pool.tile([128, FDQ], dt.float16, tag="st")
                nc.vector.transpose(st[:], dq[:].rearrange("p b i -> p (b i)"))
                return st

            def emit_dq_store(l, ci, r, cix, st):
                """GpSimd: 4 block-grid-swapped stores complete the transpose.
                dst[k=cix*FDQ+32b+u, h=32a+v] = st[32a+u, 32b+v]."""
                ri = r - CHUNKS[l][ci][0]
                for a in range(4):
                    nc.gpsimd.dma_start(
                        dq_shard[(l, ci)][ri, cix * FDQ:(cix + 1) * FDQ,
                                          32 * a:32 * (a + 1)]
                        .rearrange("(f p) h -> p f h", p=32),
                        st[32 * a:32 * (a + 1), :]
                        .rearrange("p (f i) -> p f i", i=32))

            def emit_ag(l, ci):
                nc.gpsimd.collective_compute(
                    "AllGather", Alu.bypass,
                    replica_groups=[list(range(N_CORES))],
                    ins=[dq_shard[(l, ci)][:]],
                    outs=[dq_chunk[(l, ci)][:]],
                )

            # ---- event list: (ready_us, seq, kind, payload), ready-ordered ----
            events = []
            seq = 0
            ntk = {l: WDIMS[l][1] // FDQ for l in WDIMS}
            for t in all_tiles:
                i = t_index[t]
                rdy_load = 0.0 if i < 3 else dve_done(i - 3)
                events.append((rdy_load, seq, "load", t)); seq += 1
                events.append((rdy_load, seq, "vec", t)); seq += 1
                l, r, cix = t
                if cix == ntk[l] - 1:
                    ci = next(c for c, ch in enumerate(CHUNKS[l]) if r in ch)
                    if r == CHUNKS[l][ci][-1]:
                        events.append((dve_done(i) + EV_MARGIN_US, seq, "ag",
                                       (l, ci)))
                        seq += 1
            events.sort(key=lambda e: (e[0], e[1]))
            events = events[::-1]  # pop from end

            staged = {}   # tile -> dqin handle

            def chunk_of(l, r):
                return next(c for c, ch in enumerate(CHUNKS[l]) if r in ch)

            def flush_events(now_us):
                while events and events[-1][0] <= now_us:
                    _, _, kind, payload = events.pop()
                    if kind == "load":
                        staged[payload] = emit_dq_load(*payload)
                    elif kind == "vec":
                        l, r, cix = payload
                        st = emit_dq_vec(l, r, cix, staged.pop(payload))
                        emit_dq_store(l, chunk_of(l, r), r, cix, st)
                    else:
                        emit_ag(*payload)

            # everything ready before the first epilogue goes out up front
            flush_events(L1_START_US)

            # ---- matmul layers, chunk-ordered j loops ----
            t_us = L1_START_US
            for l, (dout, K) in WDIMS.items():
                nj = dout // 128
                nk = K // 128
                nrt = NRT[l]
                half = nk // 2
                out_dt = dt.float32 if l == 4 else dt.float16
                a_next = apool.tile([128, nj, BS], out_dt, tag="acts")
                j_period = nk * (BS // 512) * MM_US
                for ci, chunk in enumerate(CHUNKS[l]):
                    for c in range(N_CORES):
                        for ri, r in enumerate(chunk):
                            j = c * nrt + r
                            src = dq_chunk[(l, ci)][c * len(chunk) + ri]
                            wts = []
                            for i0 in (0, half):
                                wt_h = wpool.tile([128, half, 128], dt.float16,
                                                  tag="wt")
                                nc.sync.dma_start(
                                    wt_h[:],
                                    src[i0 * 128:(i0 + half) * 128, :]
                                    .rearrange("(i p) h -> p i h", p=128))
                                wts.append(wt_h)
                            ps = []
                            for _n in range(BS // 512):
                                ps_t = pspool.tile([128, 512], dt.float32,
                                                   tag="ps")
                                ps.append(ps_t)
                            for i in range(nk):
                                for n in range(BS // 512):
                                    nc.tensor.matmul(
                                        ps[n][:], wts[i // half][:, i % half, :],
                                        a_cur[:, i, n * 512:(n + 1) * 512],
                                        start=(i == 0), stop=(i == nk - 1))
                            t_us += j_period
                            flush_events(t_us)
                            act_fn = Act.Sigmoid if l == 4 else Act.Relu
                            for n in range(BS // 512):
                                nc.scalar.activation(
                                    a_next[:, j, n * 512:(n + 1) * 512],
                                    ps[n][:], act_fn,
                                    bias=b_sb[l][:, j:j + 1], scale=1.0)
                            if l == 4:
                                nc.sync.dma_start(
                                    y_out[j * 128:(j + 1) * 128, :],
                                    a_next[:, j, :])
                a_cur = a_next
            flush_events(1e9)

    nc.compile()
    return nc


def _get_nc():
    if "nc" not in _CACHED:
        _CACHED["nc"] = _build_nc()
    return _CACHED["nc"]


def build_in_maps(inputs):
    x = np.asarray(inputs["x"], dtype=np.float32)
    ws = {l: np.ascontiguousarray(np.asarray(inputs[f"w{l}"], dtype=np.float32))
          for l in (1, 2, 3, 4)}
    bs = {l: np.ascontiguousarray(
        np.asarray(inputs[f"b{l}"], dtype=np.float32).reshape(-1, 128).T)
        for l in (1, 2, 3, 4)}
    in_maps = []
    for c in range(N_CORES):
        m = {
            "xst": np.ascontiguousarray(
                x[c * BS:(c + 1) * BS].T.astype(np.float16)),
            "w1s": ws[1][c * HS:(c + 1) * HS],
            "w2s": ws[2][c * HS:(c + 1) * HS],
            "w3s": ws[3][c * HS:(c + 1) * HS],
            "w4s": ws[4][c * OS:(c + 1) * OS],
            "b1": bs[1], "b2": bs[2], "b3": bs[3], "b4": bs[4],
        }
        in_maps.append(m)
    return in_maps


def kernel(**inputs):
    from concourse.bass_utils import run_bass_kernel_spmd

    nc = _get_nc()
    in_maps = build_in_maps(inputs)
    res = run_bass_kernel_spmd(nc, in_maps, list(range(N_CORES)))
    out = np.empty((B, OUT), dtype=np.float32)
    for c in range(N_CORES):
        out[c * BS:(c + 1) * BS] = res.results[c]["y"].T
    return out


if __name__ == "__main__":
    rng = np.random.default_rng(0)
    ins = {
        "x": rng.standard_normal((B, IN)).astype(np.float32),
        "w1": (rng.standard_normal((H, IN)) * 0.1).astype(np.float32),
        "b1": np.zeros(H, np.float32),
        "w2": (rng.standard_normal((H, H)) * 0.1).astype(np.float32),
        "b2": np.zeros(H, np.float32),
        "w3": (rng.standard_normal((H, H)) * 0.1).astype(np.float32),
        "b3": np.zeros(H, np.float32),
        "w4": (rng.standard_normal((OUT, H)) * 0.1).astype(np.float32),
        "b4": np.zeros(OUT, np.float32),
    }
    y = kernel(**ins)
    print("kernel ran, output shape", y.shape, "mean", float(y.mean()))
